# revision 15
# baseline (speedup 1.0000x reference)
"""GQA attention (tanh-score + static bias, no softmax) on 8 trn2 cores.

Reference shapes: x [4,32,256,512], H=8 heads, G=2 kv groups, D=64, N=256.
Strategy: data-parallel over the 128 (b,t) pairs -> 16 per core, zero
collectives.  All matmul operands are bf16 (PSUM accumulation stays fp32):
bf16 streams 1 row/cycle at any free size (fp32r needs free>=256), enables
the fast weight-load path, and halves SBUF/HBM traffic.  Per (b,t):
  q = x@Wq, k = x@Wk, v = x@Wv          (feature-major via host-transposed x)
  scores^T[m,n] = k_g @ q_h^T           (K=64 contraction, base-aligned;
                                         head pair (h, h+4) shares one PSUM
                                         bank -> one 512-wide tanh ACTIVATE)
  attn^T = tanh(scores^T * 0.125)       (ACT engine, scale fused, bf16 out)
  out_h^T = v_g^T @ attn_h^T + (sgr v_g)^T   (one 512-wide matmul per head
                                              pair; sgr@v once per group,
                                              added during PSUM evacuation)
  y = out @ Wo                           (Wo host-permuted to match pair order)

Emission interleaves next-iteration projections between score/attention units
so the PE never idles waiting on the ACT (tanh) pipeline.

Host-side prep (outside the HW kernel): x transposed to feature-major,
pre-tiled and cast to bf16, sgr transposed, Wk concatenated with its
group-swapped copy, Wo row-permuted; all weights cast to bf16.
"""

import os
import sys

import numpy as np

for _p in ("/opt/trn_rl_repo",):
    if _p not in sys.path and os.path.isdir(_p):
        sys.path.insert(0, _p)

import ml_dtypes

import concourse.bass as bass
import concourse.tile as tile
from concourse import bacc, mybir
from concourse.bass_utils import run_bass_kernel_spmd

F32 = mybir.dt.float32
BF16 = mybir.dt.bfloat16

B, T, N, C = 4, 32, 256, 512
H, G, D = 8, 2, 64
NCORES = 8
BT = B * T                      # 128
PER_CORE = BT // NCORES         # 16
NPAIR = PER_CORE // 2           # 8 iterations of 2 (b,t) each
SCALE = D ** -0.5               # 0.125

_cached = {}


def _build_nc():
    """Build + lower the single-core SPMD program."""
    nc = bacc.Bacc("TRN2", target_bir_lowering=False, debug=False,
                   num_devices=NCORES)

    # DRAM I/O (per-core shard, host-side pre-arranged)
    # xarr[i, p, c, 256*b + n] = x[bt=2i+b, tok=n, cin=128c+p]
    xT = nc.dram_tensor("xT", [NPAIR, 128, 4, 512], BF16, kind="ExternalInput").ap()
    sgrT = nc.dram_tensor("sgrT", [N, N], BF16, kind="ExternalInput").ap()
    Wq = nc.dram_tensor("Wq", [C, C], BF16, kind="ExternalInput").ap()
    Wkc = nc.dram_tensor("Wkc", [C, G * D], BF16, kind="ExternalInput").ap()
    Wv = nc.dram_tensor("Wv", [C, G * D], BF16, kind="ExternalInput").ap()
    Wop = nc.dram_tensor("Wop", [C, C], BF16, kind="ExternalInput").ap()
    y = nc.dram_tensor("y", [PER_CORE, N, C], F32, kind="ExternalOutput").ap()

    with tile.TileContext(nc) as tc:
        _body(tc, xT, sgrT, Wq, Wkc, Wv, Wop, y)

    nc.compile()
    return nc


def _body(tc, xT, sgrT, Wq, Wkc, Wv, Wop, y):
    nc = tc.nc
    mm = nc.tensor.matmul
    import contextlib
    ctx = contextlib.ExitStack()
    with ctx:
        consts = ctx.enter_context(tc.tile_pool(name="consts", bufs=1))
        xpool = ctx.enter_context(tc.tile_pool(name="xt", bufs=3))
        qpool = ctx.enter_context(tc.tile_pool(name="qs", bufs=8))
        kpool = ctx.enter_context(tc.tile_pool(name="ks", bufs=4))
        vpool = ctx.enter_context(tc.tile_pool(name="vs", bufs=2))
        svpool = ctx.enter_context(tc.tile_pool(name="svs", bufs=2))
        apool = ctx.enter_context(tc.tile_pool(name="attn", bufs=26))
        ppool = ctx.enter_context(tc.tile_pool(name="pairs", bufs=10))
        ypool = ctx.enter_context(tc.tile_pool(name="ys", bufs=6))
        psA = ctx.enter_context(
            tc.tile_pool(name="psA", bufs=2, space=bass.MemorySpace.PSUM))
        psS = ctx.enter_context(
            tc.tile_pool(name="psS", bufs=3, space=bass.MemorySpace.PSUM))
        psV = ctx.enter_context(
            tc.tile_pool(name="psV", bufs=3, space=bass.MemorySpace.PSUM))

        # pipeline state: projections of the iteration currently in stage B
        # (cur) and the one being produced by interleaved stage A (new)
        xts = [None] * NPAIR

        def dma_x(it):
            if it < NPAIR:
                t = xpool.tile([128, 4, 512], BF16, tag="xt")
                nc.sync.dma_start(t[:], xT[it])
                xts[it] = t

        # ---- resident constants, issued just-in-time for stage A of iter 0:
        # x chunk c right before the wq block that consumes it, so the first
        # q matmul fires after ~0.25MB instead of after the whole prologue;
        # Wo and sgr trail (not needed until well into the first iteration).
        xt0 = xpool.tile([128, 4, 512], BF16, tag="xt")
        xts[0] = xt0
        wq = []
        wkc = []
        wv = []
        wo = []
        for c in range(4):
            nc.sync.dma_start(xt0[:, c, :], xT[0, :, c, :])
            t = consts.tile([128, 512], BF16, tag=f"wq{c}")
            nc.sync.dma_start(t[:], Wq[128 * c:128 * (c + 1), :])
            wq.append(t)
        for c in range(4):
            t = consts.tile([128, 128], BF16, tag=f"wkc{c}")
            nc.sync.dma_start(t[:], Wkc[128 * c:128 * (c + 1), :])
            wkc.append(t)
            t = consts.tile([128, 128], BF16, tag=f"wv{c}")
            nc.sync.dma_start(t[:], Wv[128 * c:128 * (c + 1), :])
            wv.append(t)
        dma_x(1)
        sgt = []
        for mc in range(2):
            t = consts.tile([128, 256], BF16, tag=f"sgt{mc}")
            nc.sync.dma_start(t[:], sgrT[128 * mc:128 * (mc + 1), :])
            sgt.append(t)
        for c in range(4):
            t = consts.tile([128, 512], BF16, tag=f"wo{c}")
            nc.sync.dma_start(t[:], Wop[128 * c:128 * (c + 1), :])
            wo.append(t)

        # ---- stage A units (projections for iteration `it`) ----
        def make_a_units(it):
            xt = xts[it]
            qs_new = [None] * 4
            ks_new = [None] * 2
            vsv = {}

            def qu(j):
                ps = psA.tile([128, 512], F32, tag="psA")
                for c in range(4):
                    mm(ps[:], wq[c][:, 128 * j:128 * (j + 1)],
                       xt[:, c, :], start=(c == 0), stop=(c == 3))
                s = qpool.tile([128, 512], BF16, tag="qs")
                nc.vector.tensor_copy(s[:], ps[:])
                qs_new[j] = s

            def ku():
                # k feature-major [g0 d; g1 d] x 512 tokens; the group-swapped
                # layout ks2 = [g1; g0] is a partition-swap SBUF->SBUF DMA
                # copy instead of a second projection (saves 4 matmuls).
                ps = psA.tile([128, 512], F32, tag="psA")
                for c in range(4):
                    mm(ps[:], wkc[c][:],
                       xt[:, c, :], start=(c == 0), stop=(c == 3))
                s = kpool.tile([128, 512], BF16, tag="ks")
                nc.vector.tensor_copy(s[:], ps[:])
                s2 = kpool.tile([128, 512], BF16, tag="ks")
                nc.sync.dma_start(s2[0:64, :], s[64:128, :])
                nc.sync.dma_start(s2[64:128, :], s[0:64, :])
                ks_new[0] = s
                ks_new[1] = s2

            def vu(k):
                # v token-major quarters: k = 2*b + mc -> [128 tok, 128 dd]
                if k == 0:
                    vsv["ps"] = psV.tile([128, 512], F32, tag="psV", name="vps")
                ps = vsv["ps"]
                b, mc = k // 2, k % 2
                off = 256 * b + 128 * mc
                for c in range(4):
                    mm(ps[:, 128 * k:128 * (k + 1)], xt[:, c, off:off + 128],
                       wv[c][:], start=(c == 0), stop=(c == 3))
                if k == 3:
                    s = vpool.tile([128, 512], BF16, tag="vs")
                    nc.vector.tensor_copy(s[:], ps[:])
                    vsv["vs"] = s

            def sgru():
                # (sgr @ v)^T for both groups, per b: cols 256*b..256*b+255
                vs_new = vsv["vs"]
                ps = psV.tile([128, 512], F32, tag="psV")
                for b in range(2):
                    for mc in range(2):
                        mm(ps[:, 256 * b:256 * (b + 1)],
                           vs_new[:, 128 * (2 * b + mc):128 * (2 * b + mc + 1)],
                           sgt[mc][:], start=(mc == 0), stop=(mc == 1))
                s = svpool.tile([128, 512], F32, tag="svs")
                nc.vector.tensor_copy(s[:], ps[:])
                vsv["svs"] = s

            return qu, ku, vu, sgru, qs_new, ks_new, vsv

        state = [None] * (NPAIR + 1)

        def stage_a_plain(it):
            qu, ku, vu, sgru, qs_new, ks_new, vsv = make_a_units(it)
            for j in range(4):
                qu(j)
            ku()
            for k in range(4):
                vu(k)
            sgru()
            state[it] = (qs_new, ks_new, vsv)

        attn_store = {}

        def get_attn(it):
            if it not in attn_store:
                attn_store[it] = [[[None, None] for _ in range(4)]
                                  for _ in range(2)]
            return attn_store[it]

        def make_su(it, qs, ks, attn):
            def su(i):
                # scores^T + tanh for head pair (p, p+4): one PSUM bank,
                # one 512-wide ACTIVATE
                b, p, mc = i // 8, (i % 8) // 2, i % 2
                half = p % 2
                r0, r1 = 64 * half, 64 * (half + 1)
                off = 256 * b + 128 * mc
                ps = psS.tile([128, 512], F32, tag="psS", name="sps")
                mm(ps[:, 0:256], ks[half][r0:r1, off:off + 128],
                   qs[p // 2][r0:r1, 256 * b:256 * (b + 1)],
                   start=True, stop=True)
                mm(ps[:, 256:512], ks[1 - half][r0:r1, off:off + 128],
                   qs[p // 2 + 2][r0:r1, 256 * b:256 * (b + 1)],
                   start=True, stop=True)
                a = apool.tile([128, 512], BF16, tag="attn", name="attn")
                nc.scalar.activation(
                    a[:], ps[:], mybir.ActivationFunctionType.Tanh,
                    scale=SCALE)
                attn[b][p][mc] = a
            return su

        def emit_iter(it):
            """Stage B of iteration `it`, interleaved with stage A of it+1."""
            qs, ks, vsv_cur = state[it]
            vs_cur = vsv_cur["vs"]
            svs_cur = vsv_cur["svs"]
            has_next = it + 1 < NPAIR
            dma_x(it + 2)
            if has_next:
                qu, ku, vu, sgru, qs_new, ks_new, vsv_new = make_a_units(it + 1)
                big = [lambda j=j: qu(j) for j in range(4)] + [ku, lambda: None]
                small = [lambda k=k: vu(k) for k in range(4)] + [sgru]

            attn = get_attn(it)
            pairs = [[None] * 4 for _ in range(2)]
            su = make_su(it, qs, ks, attn)

            def av(j):
                # (attn @ v)^T for the head pair, both m-chunks accumulated;
                # rows 0:64 x cols 0:256 belong to head p (group 0), rows
                # 64:128 x cols 256:512 to head p+4 (group 1).  sgr@v is
                # added during the PSUM->SBUF evacuation.
                b, p = j // 4, j % 4
                ps = psV.tile([128, 512], F32, tag="psV")
                for mc in range(2):
                    mm(ps[:], vs_cur[:, 128 * (2 * b + mc):128 * (2 * b + mc + 1)],
                       attn[b][p][mc][:], start=(mc == 0), stop=(mc == 1))
                s = ppool.tile([128, 256], BF16, tag="pairs")
                nc.vector.tensor_add(s[0:64, :], ps[0:64, 0:256],
                                     svs_cur[0:64, 256 * b:256 * (b + 1)])
                nc.vector.tensor_add(s[64:128, :], ps[64:128, 256:512],
                                     svs_cur[64:128, 256 * b:256 * (b + 1)])
                pairs[b][p] = s

            def out(b, tc_):
                ps = psA.tile([128, 512], F32, tag="psA")
                for p in range(4):
                    mm(ps[:], pairs[b][p][:, 128 * tc_:128 * (tc_ + 1)],
                       wo[p][:], start=(p == 0), stop=(p == 3))
                s = ypool.tile([128, 512], F32, tag="ys")
                nc.scalar.copy(s[:], ps[:])
                nc.sync.dma_start(
                    y[2 * it + b, 128 * tc_:128 * (tc_ + 1), :], s[:])

            # interleaved emission: scores feed ACT early; projection matmuls
            # of it+1 fill the PE while ACT drains; attn@v follows tanh.
            if has_next:
                for i in range(6):
                    su(i)
                    big[i]()
                su(6); av(0)
                su(7); small[0]()
                su(8); av(1)
                su(9); small[1]()
                su(10); av(2)
                su(11); small[2]()
                su(12); av(3)
                su(13); small[3]()
                su(14); small[4]()
                su(15)
                if it + 1 == NPAIR - 1:
                    # second-to-last: pre-emit the first half of the last
                    # iteration's score units so the tail has tanh results
                    # ready and the ACT pipeline never starves the PE.
                    su_nxt = make_su(it + 1, qs_new, ks_new, get_attn(it + 1))
                    av(4); su_nxt(0)
                    av(5); su_nxt(1)
                    av(6); su_nxt(2)
                    av(7); su_nxt(3)
                    out(0, 0); su_nxt(4)
                    out(0, 1); su_nxt(5)
                    out(1, 0); su_nxt(6)
                    out(1, 1); su_nxt(7)
                else:
                    av(4); av(5); av(6); av(7)
                    out(0, 0); out(0, 1); out(1, 0); out(1, 1)
            else:
                # tail: first-half scores were pre-emitted by the previous
                # iteration; interleave the rest with attn@v and out-proj.
                av(0); su(8)
                av(1); su(9)
                av(2); su(10)
                av(3); su(11)
                out(0, 0); su(12)
                out(0, 1); su(13)
                av(4); su(14)
                av(5); su(15)
                av(6); av(7)
                out(1, 0); out(1, 1)

            state[it] = None
            if has_next:
                state[it + 1] = (qs_new, ks_new, vsv_new)

        dma_x(0)
        dma_x(1)
        stage_a_plain(0)
        for it in range(NPAIR):
            emit_iter(it)


def _get_runner():
    if "nc" not in _cached:
        _cached["nc"] = _build_nc()
    return _cached["nc"]


def _prep_inputs(x, sgr, Wq, Wk, Wv, Wo):
    bf16 = ml_dtypes.bfloat16
    x = np.ascontiguousarray(x, dtype=np.float32)
    xb = x.reshape(BT, N, C)
    Wkc = np.ascontiguousarray(np.asarray(Wk, dtype=np.float32)).astype(bf16)
    # Wo rows permuted to pair order [h0,h4 | h1,h5 | h2,h6 | h3,h7]
    perm = np.concatenate(
        [np.r_[64 * p:64 * (p + 1), 64 * (p + 4):64 * (p + 5)]
         for p in range(4)])
    Wop = np.ascontiguousarray(
        np.asarray(Wo, dtype=np.float32)[perm, :]).astype(bf16)
    sgrT = np.ascontiguousarray(
        np.asarray(sgr, dtype=np.float32).T).astype(bf16)
    Wq = np.ascontiguousarray(np.asarray(Wq, dtype=np.float32)).astype(bf16)
    Wv = np.ascontiguousarray(np.asarray(Wv, dtype=np.float32)).astype(bf16)

    in_maps = []
    for core in range(NCORES):
        xc = xb[PER_CORE * core: PER_CORE * (core + 1)]        # [16, 256, 512]
        xtc = xc.transpose(0, 2, 1)                            # [16, 512, 256]
        xarr = np.ascontiguousarray(
            xtc.reshape(NPAIR, 2, 4, 128, N)
               .transpose(0, 3, 2, 1, 4)
               .reshape(NPAIR, 128, 4, 512)).astype(bf16)
        in_maps.append({
            "xT": xarr, "sgrT": sgrT, "Wq": Wq, "Wkc": Wkc,
            "Wv": Wv, "Wop": Wop,
        })
    return in_maps


def _run(x, sgr, Wq, Wk, Wv, Wo, trace=False, tmpdir=None):
    nc = _get_runner()
    in_maps = _prep_inputs(x, sgr, Wq, Wk, Wv, Wo)
    res = run_bass_kernel_spmd(nc, in_maps, list(range(NCORES)), trace=trace,
                               tmpdir=tmpdir)
    outs = [res.results[i]["y"] for i in range(NCORES)]
    full = np.concatenate(outs, axis=0).reshape(B, T, N, C)
    return full, res


def kernel(x, sgr, Wq, Wk, Wv, Wo):
    out, _ = _run(x, sgr, Wq, Wk, Wv, Wo, trace=False)
    return out


# revision 21
# speedup vs baseline: 1.0624x; 1.0624x over previous
"""GQA attention (tanh-score + static bias, no softmax) on 8 trn2 cores.

Reference shapes: x [4,32,256,512], H=8 heads, G=2 kv groups, D=64, N=256.
Strategy: data-parallel over the 128 (b,t) pairs -> 16 per core, zero
collectives.  All matmul operands are bf16 (PSUM accumulation stays fp32):
bf16 streams 1 row/cycle at any free size (fp32r needs free>=256), enables
the fast weight-load path, and halves SBUF/HBM traffic.  Per (b,t):
  q = x@Wq, k = x@Wk, v = x@Wv          (feature-major via host-transposed x)
  scores^T[m,n] = k_g @ q_h^T           (K=64 contraction, base-aligned;
                                         head pair (h, h+4) shares one PSUM
                                         bank -> one 512-wide tanh ACTIVATE)
  attn^T = tanh(scores^T * 0.125)       (ACT engine, scale fused, bf16 out)
  out_h^T = v_g^T @ attn_h^T + (sgr v_g)^T   (one 512-wide matmul per head
                                              pair; sgr@v once per group,
                                              added during PSUM evacuation)
  y = out @ Wo                           (Wo host-permuted to match pair order)

Emission interleaves next-iteration projections between score/attention units
so the PE never idles waiting on the ACT (tanh) pipeline.

Host-side prep (outside the HW kernel): x transposed to feature-major,
pre-tiled and cast to bf16, sgr transposed, Wk concatenated with its
group-swapped copy, Wo row-permuted; all weights cast to bf16.
"""

import os
import sys

import numpy as np

for _p in ("/opt/trn_rl_repo",):
    if _p not in sys.path and os.path.isdir(_p):
        sys.path.insert(0, _p)

import ml_dtypes

import concourse.bass as bass
import concourse.tile as tile
from concourse import bacc, mybir
from concourse.bass_utils import run_bass_kernel_spmd

F32 = mybir.dt.float32
BF16 = mybir.dt.bfloat16

B, T, N, C = 4, 32, 256, 512
H, G, D = 8, 2, 64
NCORES = 8
BT = B * T                      # 128
PER_CORE = BT // NCORES         # 16
NPAIR = PER_CORE // 2           # 8 iterations of 2 (b,t) each
SCALE = D ** -0.5               # 0.125

_cached = {}


def _build_nc():
    """Build + lower the single-core SPMD program."""
    nc = bacc.Bacc("TRN2", target_bir_lowering=False, debug=False,
                   num_devices=NCORES)

    # DRAM I/O (per-core shard, host-side pre-arranged)
    # xarr[i, p, c, 256*b + n] = x[bt=2i+b, tok=n, cin=128c+p]
    xT = nc.dram_tensor("xT", [NPAIR, 128, 4, 512], BF16, kind="ExternalInput").ap()
    # all weights packed into one tensor -> 2 DMA dispatches instead of 18
    # (each dma_start costs ~0.6us of serialized sync-queue dispatch time):
    # cols [0:2048 wq | 2048:2560 wkc | 2560:3072 wv | 3072:3584 sgt |
    #       3584:5632 wo]
    Wpk = nc.dram_tensor("Wpk", [128, 5632], BF16, kind="ExternalInput").ap()
    y = nc.dram_tensor("y", [PER_CORE, N, C], F32, kind="ExternalOutput").ap()

    with tile.TileContext(nc) as tc:
        _body(tc, xT, Wpk, y)

    nc.compile()
    return nc


def _body(tc, xT, Wpk, y):
    nc = tc.nc
    mm = nc.tensor.matmul
    import contextlib
    ctx = contextlib.ExitStack()
    with ctx:
        consts = ctx.enter_context(tc.tile_pool(name="consts", bufs=1))
        xpool = ctx.enter_context(tc.tile_pool(name="xt", bufs=3))
        qpool = ctx.enter_context(tc.tile_pool(name="qs", bufs=8))
        kpool = ctx.enter_context(tc.tile_pool(name="ks", bufs=4))
        vpool = ctx.enter_context(tc.tile_pool(name="vs", bufs=2))
        svpool = ctx.enter_context(tc.tile_pool(name="svs", bufs=2))
        apool = ctx.enter_context(tc.tile_pool(name="attn", bufs=26))
        ppool = ctx.enter_context(tc.tile_pool(name="pairs", bufs=10))
        ypool = ctx.enter_context(tc.tile_pool(name="ys", bufs=6))
        psA = ctx.enter_context(
            tc.tile_pool(name="psA", bufs=2, space=bass.MemorySpace.PSUM))
        psS = ctx.enter_context(
            tc.tile_pool(name="psS", bufs=4, space=bass.MemorySpace.PSUM))
        psV = ctx.enter_context(
            tc.tile_pool(name="psV", bufs=2, space=bass.MemorySpace.PSUM))

        # pipeline state: projections of the iteration currently in stage B
        # (cur) and the one being produced by interleaved stage A (new)
        xts = [None] * NPAIR

        def dma_x(it):
            if it < NPAIR:
                t = xpool.tile([128, 4, 512], BF16, tag="xt")
                nc.sync.dma_start(t[:], xT[it])
                xts[it] = t

        # ---- resident constants: one packed tile, two dma_starts (the
        # second covers sgr+Wo which aren't needed until later), with the
        # x loads for iterations 0/1 dispatched in parallel.
        dma_x(0)
        wpk = consts.tile([128, 5632], BF16, tag="wpk")
        nc.sync.dma_start(wpk[:, 0:3072], Wpk[:, 0:3072])
        dma_x(1)
        nc.sync.dma_start(wpk[:, 3072:5632], Wpk[:, 3072:5632])
        wq = [wpk[:, 512 * c:512 * (c + 1)] for c in range(4)]
        wkc = [wpk[:, 2048 + 128 * c:2048 + 128 * (c + 1)] for c in range(4)]
        wv = [wpk[:, 2560 + 128 * c:2560 + 128 * (c + 1)] for c in range(4)]
        sgt = [wpk[:, 3072 + 256 * mc:3072 + 256 * (mc + 1)] for mc in range(2)]
        wo = [wpk[:, 3584 + 512 * c:3584 + 512 * (c + 1)] for c in range(4)]

        # ---- stage A units (projections for iteration `it`) ----
        def make_a_units(it):
            xt = xts[it]
            qs_new = [None] * 4
            ks_new = [None] * 2
            vsv = {}

            def qu(j):
                ps = psA.tile([128, 512], F32, tag="psA")
                for c in range(4):
                    mm(ps[:], wq[c][:, 128 * j:128 * (j + 1)],
                       xt[:, c, :], start=(c == 0), stop=(c == 3))
                s = qpool.tile([128, 512], BF16, tag="qs")
                nc.vector.tensor_copy(s[:], ps[:])
                qs_new[j] = s

            def ku():
                # k feature-major [g0 d; g1 d] x 512 tokens; the group-swapped
                # layout ks2 = [g1; g0] is a partition-swap SBUF->SBUF DMA
                # copy instead of a second projection (saves 4 matmuls).
                ps = psA.tile([128, 512], F32, tag="psA")
                for c in range(4):
                    mm(ps[:], wkc[c][:],
                       xt[:, c, :], start=(c == 0), stop=(c == 3))
                s = kpool.tile([128, 512], BF16, tag="ks")
                nc.vector.tensor_copy(s[:], ps[:])
                s2 = kpool.tile([128, 512], BF16, tag="ks")
                nc.sync.dma_start(s2[0:64, :], s[64:128, :])
                nc.sync.dma_start(s2[64:128, :], s[0:64, :])
                ks_new[0] = s
                ks_new[1] = s2

            def vu(k):
                # v token-major quarters: k = 2*b + mc -> [128 tok, 128 dd]
                if k == 0:
                    vsv["ps"] = psV.tile([128, 512], F32, tag="psV", name="vps")
                ps = vsv["ps"]
                b, mc = k // 2, k % 2
                off = 256 * b + 128 * mc
                for c in range(4):
                    mm(ps[:, 128 * k:128 * (k + 1)], xt[:, c, off:off + 128],
                       wv[c][:], start=(c == 0), stop=(c == 3))
                if k == 3:
                    s = vpool.tile([128, 512], BF16, tag="vs")
                    nc.vector.tensor_copy(s[:], ps[:])
                    vsv["vs"] = s

            def sgru():
                # (sgr @ v)^T for both groups, per b: cols 256*b..256*b+255
                vs_new = vsv["vs"]
                ps = psV.tile([128, 512], F32, tag="psV")
                for b in range(2):
                    for mc in range(2):
                        mm(ps[:, 256 * b:256 * (b + 1)],
                           vs_new[:, 128 * (2 * b + mc):128 * (2 * b + mc + 1)],
                           sgt[mc][:], start=(mc == 0), stop=(mc == 1))
                s = svpool.tile([128, 512], F32, tag="svs")
                nc.vector.tensor_copy(s[:], ps[:])
                vsv["svs"] = s

            return qu, ku, vu, sgru, qs_new, ks_new, vsv

        state = [None] * (NPAIR + 1)

        def stage_a_plain(it):
            qu, ku, vu, sgru, qs_new, ks_new, vsv = make_a_units(it)
            for j in range(4):
                qu(j)
            ku()
            for k in range(4):
                vu(k)
            sgru()
            state[it] = (qs_new, ks_new, vsv)

        attn_store = {}

        def get_attn(it):
            if it not in attn_store:
                attn_store[it] = [[[None, None] for _ in range(4)]
                                  for _ in range(2)]
            return attn_store[it]

        def make_su(it, qs, ks, attn):
            def su(i):
                # scores^T + tanh for head pair (p, p+4): one PSUM bank,
                # one 512-wide ACTIVATE
                b, p, mc = i // 8, (i % 8) // 2, i % 2
                half = p % 2
                r0, r1 = 64 * half, 64 * (half + 1)
                off = 256 * b + 128 * mc
                ps = psS.tile([128, 512], F32, tag="psS", name="sps")
                mm(ps[:, 0:256], ks[half][r0:r1, off:off + 128],
                   qs[p // 2][r0:r1, 256 * b:256 * (b + 1)],
                   start=True, stop=True)
                mm(ps[:, 256:512], ks[1 - half][r0:r1, off:off + 128],
                   qs[p // 2 + 2][r0:r1, 256 * b:256 * (b + 1)],
                   start=True, stop=True)
                a = apool.tile([128, 512], BF16, tag="attn", name="attn")
                nc.scalar.activation(
                    a[:], ps[:], mybir.ActivationFunctionType.Tanh,
                    scale=SCALE)
                attn[b][p][mc] = a
            return su

        def emit_iter(it):
            """Stage B of iteration `it`, interleaved with stage A of it+1."""
            qs, ks, vsv_cur = state[it]
            vs_cur = vsv_cur["vs"]
            svs_cur = vsv_cur["svs"]
            has_next = it + 1 < NPAIR
            dma_x(it + 2)
            if has_next:
                qu, ku, vu, sgru, qs_new, ks_new, vsv_new = make_a_units(it + 1)
                big = [lambda j=j: qu(j) for j in range(4)] + [ku, lambda: None]
                small = [lambda k=k: vu(k) for k in range(4)] + [sgru]

            attn = get_attn(it)
            pairs = [[None] * 4 for _ in range(2)]
            su = make_su(it, qs, ks, attn)

            def av(j):
                # (attn @ v)^T for the head pair, both m-chunks accumulated;
                # rows 0:64 x cols 0:256 belong to head p (group 0), rows
                # 64:128 x cols 256:512 to head p+4 (group 1).  sgr@v is
                # added during the PSUM->SBUF evacuation.
                b, p = j // 4, j % 4
                ps = psV.tile([128, 512], F32, tag="psV")
                for mc in range(2):
                    mm(ps[:], vs_cur[:, 128 * (2 * b + mc):128 * (2 * b + mc + 1)],
                       attn[b][p][mc][:], start=(mc == 0), stop=(mc == 1))
                s = ppool.tile([128, 256], BF16, tag="pairs")
                nc.vector.tensor_add(s[0:64, :], ps[0:64, 0:256],
                                     svs_cur[0:64, 256 * b:256 * (b + 1)])
                nc.vector.tensor_add(s[64:128, :], ps[64:128, 256:512],
                                     svs_cur[64:128, 256 * b:256 * (b + 1)])
                pairs[b][p] = s

            def out(b, tc_):
                ps = psA.tile([128, 512], F32, tag="psA")
                for p in range(4):
                    mm(ps[:], pairs[b][p][:, 128 * tc_:128 * (tc_ + 1)],
                       wo[p][:], start=(p == 0), stop=(p == 3))
                s = ypool.tile([128, 512], F32, tag="ys")
                nc.scalar.copy(s[:], ps[:])
                nc.sync.dma_start(
                    y[2 * it + b, 128 * tc_:128 * (tc_ + 1), :], s[:])

            # Blocked emission: score units in runs of 4 (PE pays a shape-
            # transition penalty of ~100-300ns entering/leaving the K=64
            # score matmuls, so batch them), with it+1's projection matmuls
            # as filler between blocks so ACT never starves the PE.
            if has_next:
                fillers = [
                    lambda: (big[0](), big[1]()),
                    lambda: (big[2](), big[3]()),
                    lambda: (big[4](), small[0](), small[1]()),
                    lambda: (small[2](), small[3](), small[4]()),
                ]
                for blk in range(4):
                    for i in range(4):
                        su(4 * blk + i)
                    fillers[blk]()
                if it + 1 == NPAIR - 1:
                    # second-to-last: pre-emit the first half of the last
                    # iteration's score units so the tail has tanh results
                    # ready and the ACT pipeline never starves the PE.
                    su_nxt = make_su(it + 1, qs_new, ks_new, get_attn(it + 1))
                    av(0); av(1); av(2); av(3)
                    su_nxt(0); su_nxt(1); su_nxt(2); su_nxt(3)
                    av(4); av(5); av(6); av(7)
                    su_nxt(4); su_nxt(5); su_nxt(6); su_nxt(7)
                    out(0, 0); out(0, 1); out(1, 0); out(1, 1)
                else:
                    av(0); av(1); av(2); av(3)
                    av(4); av(5); av(6); av(7)
                    out(0, 0); out(0, 1); out(1, 0); out(1, 1)
            else:
                # tail: first-half scores were pre-emitted by the previous
                # iteration; interleave the rest with attn@v and out-proj.
                av(0); av(1)
                su(8); su(9); su(10); su(11)
                av(2); av(3)
                out(0, 0); out(0, 1)
                su(12); su(13); su(14); su(15)
                av(4); av(5); av(6); av(7)
                out(1, 0); out(1, 1)

            state[it] = None
            if has_next:
                state[it + 1] = (qs_new, ks_new, vsv_new)

        dma_x(0)
        dma_x(1)
        stage_a_plain(0)
        for it in range(NPAIR):
            emit_iter(it)


def _get_runner():
    if "nc" not in _cached:
        _cached["nc"] = _build_nc()
    return _cached["nc"]


def _prep_inputs(x, sgr, Wq, Wk, Wv, Wo):
    bf16 = ml_dtypes.bfloat16
    x = np.ascontiguousarray(x, dtype=np.float32)
    xb = x.reshape(BT, N, C)
    # Wo rows permuted to pair order [h0,h4 | h1,h5 | h2,h6 | h3,h7]
    perm = np.concatenate(
        [np.r_[64 * p:64 * (p + 1), 64 * (p + 4):64 * (p + 5)]
         for p in range(4)])
    Wop = np.asarray(Wo, dtype=np.float32)[perm, :]
    sgrT = np.asarray(sgr, dtype=np.float32).T

    def rows_to_chunks(w):
        # [128*nc, F] -> [128, nc*F]: row-chunk c at cols [c*F:(c+1)*F]
        nc_, rem = divmod(w.shape[0], 128)
        assert rem == 0
        return w.reshape(nc_, 128, w.shape[1]).transpose(1, 0, 2).reshape(
            128, nc_ * w.shape[1])

    Wpk = np.concatenate([
        rows_to_chunks(np.asarray(Wq, dtype=np.float32)),   # [128, 2048]
        rows_to_chunks(np.asarray(Wk, dtype=np.float32)),   # [128, 512]
        rows_to_chunks(np.asarray(Wv, dtype=np.float32)),   # [128, 512]
        rows_to_chunks(sgrT),                               # [128, 512]
        rows_to_chunks(Wop),                                # [128, 2048]
    ], axis=1).astype(bf16)
    Wpk = np.ascontiguousarray(Wpk)

    in_maps = []
    for core in range(NCORES):
        xc = xb[PER_CORE * core: PER_CORE * (core + 1)]        # [16, 256, 512]
        xtc = xc.transpose(0, 2, 1)                            # [16, 512, 256]
        xarr = np.ascontiguousarray(
            xtc.reshape(NPAIR, 2, 4, 128, N)
               .transpose(0, 3, 2, 1, 4)
               .reshape(NPAIR, 128, 4, 512)).astype(bf16)
        in_maps.append({"xT": xarr, "Wpk": Wpk})
    return in_maps


def _run(x, sgr, Wq, Wk, Wv, Wo, trace=False, tmpdir=None):
    nc = _get_runner()
    in_maps = _prep_inputs(x, sgr, Wq, Wk, Wv, Wo)
    res = run_bass_kernel_spmd(nc, in_maps, list(range(NCORES)), trace=trace,
                               tmpdir=tmpdir)
    outs = [res.results[i]["y"] for i in range(NCORES)]
    full = np.concatenate(outs, axis=0).reshape(B, T, N, C)
    return full, res


def kernel(x, sgr, Wq, Wk, Wv, Wo):
    out, _ = _run(x, sgr, Wq, Wk, Wv, Wo, trace=False)
    return out
